# revision 61
# baseline (speedup 1.0000x reference)
"""Causal self-attention (B=2, T=2048, C=1024, H=16) on 8 TRN2 NeuronCores.

Sharding: core c -> batch b = c // 4, head group = heads [4*(c%4) .. 4*(c%4)+4).
Each core computes qkv for its 4 heads on its batch, causal attention, and a
row-parallel partial of the output projection (over its 256 head channels).
The host sums the 4 partials per batch; b_proj/4 is added on-device so the sum
reproduces a single b_proj add.

All device tensors are pre-transposed on the host so the kernel never
transposes on-chip:
  xt   [C, T]    = x[b].T                     (bf16)
  wqkt [C, 512]  = w_attn[qk rows].T          (bf16)  cols: q_h0 q_h1 q_h2 q_h3 k_h0..k_h3
  wvt  [C, 256]  = w_attn[v rows].T           (bf16)
  wpt  [256, C]  = w_proj[:, head cols].T     (bf16)
  out_t[C, T]    = partial (x @ w_proj.T).T   (bf16; summed in fp32 on host)

On-chip dataflow (per head pair, row/col layouts chosen so the TensorE
contraction dim is always the partition dim and no transposes are needed):
  qT,kT [d, t] -> S^T[tk, tq] (two heads packed in the 128-wide array via
  row tiling) -> exp on ScalarE (scale=1/8 folded in) -> causal mask via
  static 0/1 mask multiply on DVE -> AV matmul with V augmented by a ones
  column (denominator accumulates in row 64 of PSUM for free) -> reciprocal +
  K=2 broadcast matmul to spread 1/den across 64 partitions -> normalize ->
  projection (stays transposed).
"""

import os
import sys
import types

import numpy as np
import ml_dtypes

import concourse.bass as bass
import concourse.mybir as mybir
import concourse.tile as tile
from concourse import bacc
from concourse.hw_specs import get_activation_tables

BF16 = ml_dtypes.bfloat16


class _Bacc(bacc.Bacc):
    """Bacc that steers Exp/Ln activations to the combined
    natural_log_exp_and_others table set so the kernel never swaps
    activation tables (set ids keep their act_info.json positions)."""

    def insert_act_table_loads(self):
        import bass_rust as _br
        import concourse.mybir as _mybir

        has_activation = any(
            isinstance(i, _mybir.InstActivation)
            for b in self.main_func.blocks
            for i in b.instructions
        )
        if not has_activation:
            return
        combined = {"natural_log_exp_and_others"}
        steer = {_mybir.ActivationFunctionType.Exp, _mybir.ActivationFunctionType.Ln}
        tables = []
        for name, fns in get_activation_tables(self.m.arch).items():
            if name not in combined:
                fns = {f for f in fns if f not in steer}
            tables.append((name, set(fns)))
        _br.insert_act_table_loads(self, tables)

B, T, C = 2, 2048, 1024
H = 16
DH = 64
N_CORES = 8
HEADS_PER_CORE = 4
TQ = 512          # tq tile (moving dim of scores/AV matmuls)
TK = 128          # tk tile (PSUM partition dim of S^T)
NG = T // TQ      # 4 tq tiles
NKT = T // TK     # 16 tk tiles
NC_ = C // 128    # 8 contraction tiles for the qkv matmuls
FP32 = mybir.dt.float32
BF16_DT = mybir.dt.bfloat16
import os as _os
DEPTH = int(_os.environ.get("K_DEPTH", "5"))
POPS_EVERY = int(_os.environ.get("K_POPS_EVERY", "2"))
BOUNDARY_POPS = int(_os.environ.get("K_BPOPS", "3"))
WARMUP_MMS = int(_os.environ.get("K_WARMUP", "12"))
PT_BUFS = int(_os.environ.get("K_PT_BUFS", "8"))


def _ensure_axon_hooks_stub():
    """bass_utils imports antenv.axon_hooks when trace is requested (even via
    the BASS_TRACE env var). The container's antenv stub lacks that module, so
    install a minimal one to keep the no-trace fallback path working."""
    try:
        import antenv  # noqa: F401
    except ImportError:
        return
    if "antenv.axon_hooks" in sys.modules:
        return
    try:
        import antenv.axon_hooks  # noqa: F401
        return
    except ImportError:
        pass
    mod = types.ModuleType("antenv.axon_hooks")
    mod._hook = None

    def set_axon_ntff_profile_hook(h):
        mod._hook = h

    def get_axon_ntff_profile_hook():
        return mod._hook

    mod.set_axon_ntff_profile_hook = set_axon_ntff_profile_hook
    mod.get_axon_ntff_profile_hook = get_axon_ntff_profile_hook
    sys.modules["antenv.axon_hooks"] = mod
    import antenv as _a

    _a.axon_hooks = mod


def build_bass():
    """Emit the single-core SPMD Bass module (same program on all 8 cores).

    Round 3 on top of the round-2 software pipeline: consumption-ordered
    input DMA on both HWDGE queues (no SWDGE bulk path), bf16 output
    partials (halves output traffic), a shared 2-bank PSUM pingpong for
    qkv/proj/broadcast matmuls (kills the single-bank WAR stall), and
    normalization at unit end so the Ln/Exp run ahead of the next unit's
    exps on the ACT queue and the av banks recycle in time.
    """
    from collections import deque
    from contextlib import ExitStack

    nc = _Bacc("TRN2", target_bir_lowering=False, debug=False)

    # All inputs arrive pre-shuffled on the host into [128, n] layouts so
    # every device DMA is a plain contiguous 2D transfer (large packets, one
    # cheap trigger each).
    xtm = nc.declare_dram_parameter("xtm", [128, 4 * 4096], BF16_DT, isOutput=False).ap()
    wqkm = nc.declare_dram_parameter("wqkm", [128, 4096], BF16_DT, isOutput=False).ap()
    wvm = nc.declare_dram_parameter("wvm", [128, 2048], BF16_DT, isOutput=False).ap()
    wpm = nc.declare_dram_parameter("wpm", [128, 2048], BF16_DT, isOutput=False).ap()
    biasm = nc.declare_dram_parameter("biasm", [128, 268], FP32, isOutput=False).ap()
    out_t = nc.declare_dram_parameter("out_t", [C, T], BF16_DT, isOutput=True).ap()

    Exp = mybir.ActivationFunctionType.Exp
    mult = mybir.AluOpType.mult
    add = mybir.AluOpType.add
    is_ge = mybir.AluOpType.is_ge

    with tile.TileContext(nc) as tc, ExitStack() as ctx:
        res = ctx.enter_context(tc.tile_pool(name="resident", bufs=1))

        # --- resident loads -------------------------------------------------
        # Few BIG transfers: each DMA trigger costs ~0.6us of issue time on
        # its engine's queue, so small per-ct transfers make the input stream
        # trigger-paced (~170 GB/s) instead of BW-paced (~350 GB/s).  All
        # layout shuffles happen on the host; here it's 8 contiguous copies.
        #   sync   — xt g0 (split in 2 for an earlier prologue start), g1..g3
        #   scalar — wqk, wv, biases, wp (its 4 trigger instructions retire
        #            before the first exp needs ACT)
        xt_g = [res.tile([128, 8 * TQ], BF16_DT, tag=f"xtg{g}", name=f"xtg{g}")
                for g in range(NG)]
        wqk_all = res.tile([128, 8 * 512], BF16_DT, tag="wqk", name="wqk_all")
        wv_all = res.tile([128, 8 * 256], BF16_DT, tag="wv", name="wv_all")
        bias_bt = res.tile([128, 268], FP32, tag="biasb", name="bias_bt")
        bqk_t = [bias_bt[:, j : j + 1] for j in range(4)]
        bp_t = [bias_bt[:, 4 + j : 5 + j] for j in range(8)]
        bv_t = bias_bt[:, 12:268]
        wp_all = res.tile([128, 2048], BF16_DT, tag="wp", name="wp_all")
        wp_t = [wp_all[:, 1024 * i : 1024 * (i + 1)] for i in range(2)]

        # xt g0 in 4 chunks and wqk in 2 halves: transfer-completion latency
        # is set by the per-engine stripe (~360ns/packet, size/16 per
        # engine), so the tiles that gate the first matmuls ship small.
        # The DMA queues serve their queued transfers ROUND-ROBIN, so a
        # gating transfer completes in (size x in-flight count)/rate time.
        # Only the transfers that gate the first matmuls go out now; the
        # bulk is released later through 1-column dummy copies that chain
        # each transfer behind its predecessor's completion (see the
        # chain releases after the warm-up block and in the unit loop).
        nc.sync.dma_start(xt_g[0][:], xtm[:, 0:4096])
        nc.scalar.dma_start(bias_bt[:], biasm[:])
        nc.scalar.dma_start(wqk_all[:, 0:2048], wqkm[:, 0:2048])

        # Single causal strip mask [128, 128]: keep iff local tq >= local tk.
        maskd = res.tile([128, 128], BF16_DT, tag="maskd", name="maskd")
        nc.gpsimd.memset(maskd[:], 1.0)
        nc.gpsimd.affine_select(
            out=maskd[:], in_=maskd[:], compare_op=is_ge, fill=0.0,
            base=0, pattern=[[1, 128]], channel_multiplier=-1,
        )

        # Ones row (lane 64, matching the av_* denominator row) for the K=1
        # broadcast matmuls.
        ones_t = res.tile([65, 64], BF16_DT, tag="ones_t", name="ones_t")
        nc.vector.memset(ones_t[:], 1.0)

        # qT/kT in [head-channel, t] layout: tile p holds heads (2p, 2p+1).
        qk_sb = [
            res.tile([128, T], BF16_DT, tag=f"qk{i}", name=f"qk{i}") for i in range(4)
        ]
        # V natural [t, d] with a ones column after each head: 4*(64+1) cols.
        v_sb = []
        for i in range(NKT):
            t = res.tile([128, 260], BF16_DT, tag=f"v{i}", name=f"v{i}")
            nc.gpsimd.memset(
                t[:].rearrange("p (h c) -> p h c", c=65)[:, :, 64:65], 1.0
            )
            v_sb.append(t)
        att_sb = [
            res.tile([128, T], BF16_DT, tag=f"att{i}", name=f"att{i}")
            for i in range(2)
        ]

        sc_ps = ctx.enter_context(tc.tile_pool(name="sc_ps", bufs=2, space="PSUM"))
        # av_e/av_o live in ONE [65, 1024] tile spanning 2 adjacent banks so
        # the denominator Ln/Exp run as single 1024-wide ACT instructions.
        av_ps = ctx.enter_context(tc.tile_pool(name="av_ps", bufs=1, space="PSUM"))
        # single 2-bank pool shared by qkv groups, proj groups and the
        # broadcast matmuls: the pingpong removes the WAR stall where each
        # group's first matmul waited on the previous group's DVE read.
        qv_ps = ctx.enter_context(tc.tile_pool(name="qv_ps", bufs=2, space="PSUM"))
        bp_ps = qv_ps
        pt_pool = ctx.enter_context(tc.tile_pool(name="pt_pool", bufs=PT_BUFS))
        riv_pool = ctx.enter_context(tc.tile_pool(name="riv", bufs=2))
        bcs_pool = ctx.enter_context(tc.tile_pool(name="bcs", bufs=2))
        scr_pool = ctx.enter_context(tc.tile_pool(name="scr", bufs=2))
        osb_pool = ctx.enter_context(tc.tile_pool(name="osb", bufs=4))

        # --- filler work: qkv projections + output projection --------------
        emitted = set()

        def emit_qk_group(jt, g):
            # wqk halves: A = [ct: q01|k01] (jt 0/2), B = [ct: q23|k23]
            # (jt 1/3) — pair-0 weights ship first and alone gate unit (0,0).
            half, sub = jt & 1, jt >> 1
            ps = qv_ps.tile([128, 512], FP32, tag="qv", name=f"qkps{jt}_{g}")
            for ct in range(NC_):
                col = 2048 * half + 256 * ct + 128 * sub
                nc.tensor.matmul(
                    ps[:],
                    lhsT=wqk_all[:, col : col + 128],
                    rhs=xt_g[g][:, TQ * ct : TQ * (ct + 1)],
                    start=(ct == 0),
                    stop=(ct == NC_ - 1),
                )

            nc.vector.tensor_scalar(
                qk_sb[jt][:, TQ * g : TQ * (g + 1)], ps[:], bqk_t[jt][:], None,
                op0=add,
            )

        def emit_v_group(tt):
            ps = qv_ps.tile([128, 512], FP32, tag="qv", name=f"vps{tt}")
            g, lt = tt // 4, tt % 4
            for ct in range(NC_):
                nc.tensor.matmul(
                    ps[:, 0:256],
                    lhsT=xt_g[g][:, TQ * ct + 128 * lt : TQ * ct + 128 * (lt + 1)],
                    rhs=wv_all[:, 256 * ct : 256 * (ct + 1)],
                    start=(ct == 0),
                    stop=(ct == NC_ - 1),
                )

            vt = v_sb[tt]
            nc.vector.tensor_tensor(
                out=vt[:].rearrange("p (h c) -> p h c", c=65)[:, :, 0:64],
                in0=ps[:, 0:256].rearrange("p (h c) -> p h c", c=64),
                in1=bv_t[:].rearrange("p (h c) -> p h c", c=64),
                op=add,
            )

        def emit_proj_group(jt, g, pool=None, tag="qv", osb_eng="vector"):
            tqs = slice(TQ * g, TQ * (g + 1))
            pp = (pool or bp_ps).tile([128, 512], FP32, tag=tag, name=f"pj{g}{jt}")
            nc.tensor.matmul(
                pp[:], lhsT=wp_t[0][:, 128 * jt : 128 * (jt + 1)],
                rhs=att_sb[0][:, tqs], start=True, stop=False,
            )
            nc.tensor.matmul(
                pp[:], lhsT=wp_t[1][:, 128 * jt : 128 * (jt + 1)],
                rhs=att_sb[1][:, tqs], start=False, stop=True,
            )
            osb = osb_pool.tile([128, 512], BF16_DT, tag="osb", name=f"osb{g}{jt}")
            if osb_eng == "scalar":
                # epilogue only: ACT is idle there, and Identity+bias matches
                # the DVE tensor_scalar — alternating engines doubles the
                # bias-add rate on the tail's critical path.
                nc.scalar.activation(
                    osb[:], pp[:], mybir.ActivationFunctionType.Identity,
                    bias=bp_t[jt][:],
                )
            else:
                nc.vector.tensor_scalar(osb[:], pp[:], bp_t[jt][:], None, op0=add)
            nc.sync.dma_start(out_t[128 * jt : 128 * (jt + 1), tqs], osb[:])

        work_q = deque()

        # --- PE warm-up: ~5us of dense zero matmuls while the DMAs stream in,
        # so the HAM clock gate opens before real compute starts ------------
        warm_sb = res.tile([128, 512], BF16_DT, tag="warm", name="warm_sb")
        nc.vector.memset(warm_sb[:], 0.0)
        warm_ps = qv_ps.tile([128, 512], FP32, tag="qv", name="warm_ps")
        for i in range(WARMUP_MMS):
            nc.tensor.matmul(
                warm_ps[:], lhsT=warm_sb[:, 0:128], rhs=warm_sb[:],
                start=(i == 0), stop=(i == WARMUP_MMS - 1), skip_group_check=True,
            )

        # --- chained bulk-input releases (round 1) --------------------------
        # Each tensor_copy reads the tail of the predecessor transfer and
        # dirties the successor's first column, so the successor's DMA
        # trigger (next on the sync queue) waits for the predecessor to
        # finish.  DVE reaches these copies before it has real work, and
        # every wait clears before the first qk bias-add is needed.
        # both round-1 releases key off xt_g0's completion so the g0 tail
        # isn't diluted by round-robin with wv/g1
        nc.vector.tensor_copy(out=wv_all[:, 0:1], in_=xt_g[0][:, 4095:4096])
        nc.sync.dma_start(wv_all[:], wvm[:])
        nc.vector.tensor_copy(out=xt_g[1][:, 0:1], in_=xt_g[0][:, 4094:4095])
        nc.sync.dma_start(xt_g[1][:], xtm[:, 4096:8192])
        nc.vector.tensor_copy(out=wqk_all[:, 2048:2049], in_=wv_all[:, 2047:2048])
        nc.sync.dma_start(wqk_all[:, 2048:4096], wqkm[:, 2048:4096])

        # Dummy zero-matmuls to keep the PE clock gate open when real filler
        # runs dry (late units and the projection tail).
        hb_n = [0]

        def heartbeat(n=2, pool=None, tag="qv"):
            t = (pool or qv_ps).tile([128, 512], FP32, tag=tag,
                                     name=f"hb{hb_n[0]}")
            hb_n[0] += 1
            for i in range(n):
                nc.tensor.matmul(
                    t[:], lhsT=warm_sb[:, 0:128], rhs=warm_sb[:],
                    start=(i == 0), stop=(i == n - 1), skip_group_check=True,
                )

        def emit_item(item):
            if item[0] == "qk":
                emit_qk_group(item[1], item[2])
            elif item[0] == "v":
                emit_v_group(item[1])
            else:
                emit_proj_group(item[1], item[2])
            emitted.add(item)

        def pop_one(force=False):
            if work_q:
                emit_item(work_q.popleft())

        def drain_until(needed):
            for item in needed:
                while item not in emitted:
                    emit_item(work_q.popleft())

        # prologue: just q/k for unit (0, 0) — its v groups ride the queue
        # and are drained right before the first av matmul, so the first
        # scores/exps start ~3.4us earlier.
        for item in [("qk", 0, 0), ("qk", 2, 0)]:
            emit_item(item)
        work_q.extend([("v", 0), ("v", 1), ("v", 2), ("v", 3),
                       ("qk", 1, 0), ("qk", 3, 0)])
        for gg in range(1, NG):
            work_q.extend(
                [("qk", 2, gg), ("qk", 0, gg), ("qk", 3, gg), ("qk", 1, gg)]
                + [("v", 4 * gg + i) for i in range(4)]
            )

        # --- attention: software-pipelined units -----------------------------
        def norm_pre(g, p, av_pair):
            """1/den via exp(-ln(den)) on ScalarE (Ln and Exp share one
            activation table set, so no table swaps); one 1024-wide
            instruction covers both heads' denominator rows."""
            Ln = mybir.ActivationFunctionType.Ln
            lr = riv_pool.tile([65, 1024], FP32, tag="lr", name=f"lr{g}{p}")
            nc.scalar.activation(lr[64:65, :], av_pair[64:65, :], Ln)
            rb = riv_pool.tile([65, 1024], BF16_DT, tag="rb", name=f"rb{g}{p}")
            nc.scalar.activation(rb[64:65, :], lr[64:65, :], Exp, scale=-1.0)
            return rb[:, 0:512], rb[:, 512:1024]

        def norm_post(g, p, av_e, av_o, riv_e, riv_o):
            """Broadcast 1/den across 64 partitions (K=1 fp32r matmul) and
            normalize; enqueues proj work for p==1."""
            tqs = slice(TQ * g, TQ * (g + 1))
            bc_e = bp_ps.tile([64, 512], FP32, tag="qv", name=f"bce{g}{p}")
            nc.tensor.matmul(
                bc_e[:], lhsT=ones_t[64:65, :], rhs=riv_e[64:65, :],
                start=True, stop=True, tile_position=(64, 0),
            )
            bcs_e = bcs_pool.tile([64, 512], FP32, tag="bcs", name=f"bcse{g}{p}")
            nc.vector.tensor_copy(out=bcs_e[:], in_=bc_e[:])
            nc.vector.tensor_tensor(
                out=att_sb[p][0:64, tqs], in0=av_e[0:64, :], in1=bcs_e[:], op=mult
            )
            bc_o = bp_ps.tile([64, 512], FP32, tag="qv", name=f"bco{g}{p}")
            nc.tensor.matmul(
                bc_o[:], lhsT=ones_t[64:65, :], rhs=riv_o[64:65, :],
                start=True, stop=True, tile_position=(64, 0),
            )
            bcs_o = bcs_pool.tile([64, 512], FP32, tag="bcs", name=f"bcso{g}{p}")
            nc.vector.tensor_copy(out=bcs_o[:], in_=bc_o[:])
            scr = scr_pool.tile([64, 512], BF16_DT, tag="scr", name=f"scr{g}{p}")
            nc.vector.tensor_tensor(
                out=scr[:], in0=av_o[0:64, :], in1=bcs_o[:], op=mult
            )
            nc.sync.dma_start(att_sb[p][64:128, tqs], scr[:])
            if p == 1:
                work_q.extend([("proj", jt, g) for jt in range(8)])

        for g, p in [(0, 0), (0, 1), (1, 0), (1, 1), (2, 0), (2, 1),
                     (3, 0), (3, 1)]:
                nkt = 4 * (g + 1)
                h_e, h_o = 2 * p, 2 * p + 1
                q_t, k_t = qk_sb[p], qk_sb[2 + p]
                tq0 = TQ * g
                # only q/k gate the first scores; v tiles are drained just
                # before the first av matmul so the v-group burst overlaps
                # the ACT exp backlog instead of delaying the unit's scores.
                drain_until(
                    [("qk", p, g)]
                    + [("qk", 2 + p, gg) for gg in range(g + 1)]
                )
                s_tiles = {}
                p_tiles = {}
                av_e = av_o = None

                def lo_of(kt, g=g):
                    i = kt - 4 * g
                    return 128 * i if i > 0 else 0

                def scores(kt, g=g, q_t=q_t, k_t=k_t, tq0=tq0, p=p):
                    lo = lo_of(kt, g)
                    s_pair = sc_ps.tile([128, 1024], FP32, tag="sc",
                                        name=f"s{g}{p}{kt}")
                    kts = slice(128 * kt, 128 * (kt + 1))
                    rq = slice(tq0 + lo, tq0 + 512)
                    nc.tensor.matmul(
                        s_pair[:, lo:512], lhsT=k_t[0:64, kts], rhs=q_t[0:64, rq],
                        start=True, stop=True,
                    )
                    nc.tensor.matmul(
                        s_pair[:, 512 + lo : 1024], lhsT=k_t[64:128, kts],
                        rhs=q_t[64:128, rq], start=True, stop=True,
                        tile_position=(64, 0),
                    )
                    s_tiles[kt] = s_pair

                def expmask(kt, g=g, p=p):
                    lo = lo_of(kt, g)
                    s_pair = s_tiles.pop(kt)
                    p_pair = pt_pool.tile([128, 1024], BF16_DT, tag="pt",
                                          name=f"p{g}{p}{kt}")
                    s3 = s_pair[:].rearrange("p (h c) -> p h c", c=512)[:, :, lo:512]
                    p3 = p_pair[:].rearrange("p (h c) -> p h c", c=512)[:, :, lo:512]
                    nc.scalar.activation(p3, s3, Exp, scale=0.125)
                    if kt >= 4 * g:  # diagonal: mask the leading 128-wide strip
                        pm = p_pair[:].rearrange("p (h c) -> p h c", c=512)[
                            :, :, lo : lo + 128
                        ]
                        mk = maskd[:, None, 0:128].to_broadcast([128, 2, 128])
                        nc.gpsimd.tensor_tensor(out=pm, in0=pm, in1=mk, op=mult)
                    p_tiles[kt] = p_pair

                def av_mm(kt, g=g, p=p, nkt=nkt, h_e=h_e, h_o=h_o):
                    lo = lo_of(kt, g)
                    p_pair = p_tiles.pop(kt)
                    nc.tensor.matmul(
                        av_e[:, lo:512], lhsT=v_sb[kt][:, 65 * h_e : 65 * h_e + 65],
                        rhs=p_pair[:, lo:512], start=(kt == 0),
                        stop=(kt == nkt - 1), skip_group_check=True,
                    )
                    nc.tensor.matmul(
                        av_o[:, lo:512], lhsT=v_sb[kt][:, 65 * h_o : 65 * h_o + 65],
                        rhs=p_pair[:, 512 + lo : 1024], start=(kt == 0),
                        stop=(kt == nkt - 1), skip_group_check=True,
                    )

                depth = min(DEPTH, nkt)
                for kt in range(2):
                    scores(kt)
                for kt in range(2):
                    expmask(kt)
                for kt in range(2, depth):
                    scores(kt)
                    expmask(kt)
                drain_until([("v", t) for t in range(nkt)])
                av_pair = av_ps.tile([65, 1024], FP32, tag="av", name=f"av{g}{p}")
                av_e = av_pair[:, 0:512]
                av_o = av_pair[:, 512:1024]
                for kt in range(depth, nkt):
                    scores(kt)
                    expmask(kt)
                    # one filler pop per POPS_EVERY kt keeps the ACT-bound
                    # late units fed without tipping them PE-bound (the
                    # per-kt ACT slack is only ~500ns at g=3).
                    if kt % POPS_EVERY == 0:
                        if work_q:
                            pop_one()
                        elif g >= 2:
                            heartbeat(3)
                    av_mm(kt - depth)
                for i, kt in enumerate(range(nkt - depth, nkt)):
                    av_mm(kt)
                    # the drain paces at ACT's exp rate with little PE work;
                    # slip one filler group in halfway through
                    if i == 1 and work_q:
                        pop_one()
                # normalize at unit end: the Ln/Exp land on the ACT queue
                # ahead of the next unit's exps, so the av banks free up
                # before the next unit's first av matmul needs them.
                rivs = norm_pre(g, p, av_pair)
                for _ in range(BOUNDARY_POPS):
                    pop_one()
                norm_post(g, p, av_e, av_o, *rivs)
                # chained bulk-input releases, rounds 2/3: by the time DVE
                # reaches these positions the predecessor transfer is done.
                if (g, p) == (0, 0):
                    nc.vector.tensor_copy(out=xt_g[2][:, 0:1],
                                          in_=xt_g[1][:, 4095:4096])
                    nc.sync.dma_start(xt_g[2][:], xtm[:, 8192:12288])
                elif (g, p) == (0, 1):
                    nc.vector.tensor_copy(out=wp_all[:, 0:1],
                                          in_=wqk_all[:, 4095:4096])
                    nc.sync.dma_start(wp_all[:], wpm[:])
                    nc.vector.tensor_copy(out=xt_g[3][:, 0:1],
                                          in_=xt_g[2][:, 4095:4096])
                    nc.sync.dma_start(xt_g[3][:], xtm[:, 12288:16384])

        # epilogue: remaining proj groups (these can rotate through the
        # now-idle 2-bank score slots for more overlap)
        eidx = 0
        while work_q:
            item = work_q.popleft()
            if item[0] == "proj":
                # alternate the 2-bank sc and qv slots (4-deep pipeline) and
                # the DVE/ACT bias-add engines so groups flow without
                # slot-recycle or engine waits
                use_sc = eidx % 2 == 0
                emit_proj_group(item[1], item[2],
                                pool=(sc_ps if use_sc else qv_ps),
                                tag=("sc" if use_sc else "qv"),
                                osb_eng=("scalar" if item[1] % 2 else "vector"))
                emitted.add(item)
                heartbeat(2, pool=(qv_ps if use_sc else sc_ps),
                          tag=("qv" if use_sc else "sc"))
                eidx += 1
            else:
                emit_item(item)

    nc.compile()
    return nc


_NC_CACHE = None


def _get_nc():
    global _NC_CACHE
    if _NC_CACHE is None:
        _NC_CACHE = build_bass()
    return _NC_CACHE


def make_in_maps(x, w_attn, b_attn, w_proj, b_proj):
    """Host-side sharding: slice/transpose/cast/shuffle the full inputs per
    core into the [128, n] layouts the device DMAs copy verbatim."""
    x = np.asarray(x, dtype=np.float32)
    w_attn = np.asarray(w_attn, dtype=np.float32)
    b_attn = np.asarray(b_attn, dtype=np.float32)
    w_proj = np.asarray(w_proj, dtype=np.float32)
    b_proj = np.asarray(b_proj, dtype=np.float32)

    def shuffle(a, nblk):
        # [128*nblk, n] -> [128, nblk*n]: row 128*blk+p, col c -> p, n*blk+c
        n = a.shape[1]
        return np.ascontiguousarray(
            a.reshape(nblk, 128, n).transpose(1, 0, 2).reshape(128, nblk * n)
        )

    in_maps = []
    for core in range(N_CORES):
        b = core // 4
        heads = [4 * (core % 4) + i for i in range(HEADS_PER_CORE)]
        ch = np.concatenate([np.arange(h * DH, (h + 1) * DH) for h in heads])
        idx_qk = np.concatenate([ch, C + ch])
        idx_v = 2 * C + ch
        xt = np.ascontiguousarray(x[b].T).astype(BF16)  # [C, T]
        # xtm[p, 4096 g + 512 ct + c] = xt[128 ct + p, 512 g + c]
        xtm = np.ascontiguousarray(
            xt.reshape(8, 128, 4, 512).transpose(1, 2, 0, 3).reshape(128, 16384)
        )
        # wqk column order [q01 q23 k01 k23] -> halves A=[q01|k01], B=[q23|k23]
        wqk_cols = w_attn[idx_qk].T.astype(BF16)  # [C, 512]
        wqkm = np.concatenate(
            [
                shuffle(np.concatenate(
                    [wqk_cols[:, 0:128], wqk_cols[:, 256:384]], axis=1), 8),
                shuffle(np.concatenate(
                    [wqk_cols[:, 128:256], wqk_cols[:, 384:512]], axis=1), 8),
            ],
            axis=1,
        )
        wvm = shuffle(w_attn[idx_v].T.astype(BF16), 8)
        wpm = shuffle(w_proj[:, ch].T.astype(BF16), 2)
        biasm = np.concatenate(
            [
                b_attn[idx_qk].astype(np.float32).reshape(4, 128).T,
                (b_proj / 4.0).astype(np.float32).reshape(8, 128).T,
                np.tile(b_attn[idx_v].astype(np.float32)[None, :], (128, 1)),
            ],
            axis=1,
        )
        in_maps.append(
            {
                "xtm": xtm,
                "wqkm": wqkm,
                "wvm": wvm,
                "wpm": wpm,
                "biasm": np.ascontiguousarray(biasm),
            }
        )
    return in_maps


def assemble_output(results):
    # out_t partials come back bf16 [C, T]; accumulate in fp32 on host.
    out = np.zeros((B, T, C), dtype=np.float32)
    for core in range(N_CORES):
        out[core // 4] += np.asarray(results[core]["out_t"], dtype=np.float32).T
    return out


def run(inputs, trace=False, trace_cores=None, tmpdir=None):
    """Run on hardware; returns (output, BassKernelResults)."""
    _ensure_axon_hooks_stub()
    from concourse.bass_utils import run_bass_kernel_spmd

    nc = _get_nc()
    in_maps = make_in_maps(**inputs)
    kw = {}
    if trace:
        kw.update(trace=True, trace_cores=trace_cores, tmpdir=tmpdir)
    res = run_bass_kernel_spmd(nc, in_maps, core_ids=list(range(N_CORES)), **kw)
    return assemble_output(res.results), res


def kernel(x, w_attn, b_attn, w_proj, b_proj):
    out, _ = run(
        dict(x=x, w_attn=w_attn, b_attn=b_attn, w_proj=w_proj, b_proj=b_proj)
    )
    return out

